# revision 18
# baseline (speedup 1.0000x reference)
"""Trainium2 Bass kernel for nn_Attn_head_40364102648200.

The reference computes a GAT-style attention head, but applies
softmax(..., axis=1) to a [B,1,N,N] tensor whose axis 1 has size 1 —
the softmax is over a singleton axis, so the attention coefficients are
identically 1.0 and the whole N x N logits/leaky-relu machinery is dead
code (for ANY input values).  The output reduces exactly to

    S[b,o]       = sum_c W1[o,c] * (sum_n x[b,c,0,n])
    out[b,o,0,n] = elu(S[b,o])            (broadcast along n)

The real work is streaming the 32 MB input x and reducing it over n
(4M adds), then a small channel contraction.  Strategy on 8
NeuronCores (channel-sharded SPMD, no cross-core collective):

  - core k reads x[:, k*64:(k+1)*64, 0, :]  (4 MB each, 1/8 of x),
    reduces over n on the Vector engine (input DMAs split across both
    HWDGE rings), and contracts its 64 channels with its W1 shard on
    the TensorEngine -> partial S_k [256, 4]
  - the host gather step sums the eight 4 KB partials (the cross-core
    reduce), applies elu to the 1024 S values, and broadcasts along n
    to materialize the full [4, 256, 1, 4096] output.

Keeping the 4 KB combine on the host instead of an on-device AllReduce
removes the all-core barrier; each core's NEFF execution is then
independent of the others' launch skew.
"""

import numpy as np

import concourse.bacc as bacc
import concourse.mybir as mybir
import concourse.tile as tile
from concourse.bass_utils import run_bass_kernel_spmd

F32 = mybir.dt.float32

N_CORES = 8
B, C, N, O = 4, 512, 4096, 256
CSH = C // N_CORES  # 64 channels per core
ROWS = B * CSH      # 256 flattened (b, c) rows per core


def _build():
    nc = bacc.Bacc(
        "TRN2",
        target_bir_lowering=False,
        debug=False,
        num_devices=N_CORES,
    )

    xk = nc.declare_dram_parameter("xk", [ROWS, N], F32, isOutput=False)
    w1tt = nc.declare_dram_parameter("w1tt", [128, O], F32, isOutput=False)
    # Partial S^T for this core's channel shard: [o_p, m*4 + b]
    out_ext = nc.declare_dram_parameter("spart", [128, 8], F32, isOutput=True)

    # Free-dim chunk boundaries per partition-tile: big chunks first, a
    # small final chunk so the last (serial) reduce is short.  Chunk h
    # rides ring h: SP-HWDGE / ACT-HWDGE / gpsimd-SWDGE — three parallel
    # DMA queues (each HWDGE ring alone tops out near ~140 GB/s here).
    BOUNDS = [0, 1536, 3072, 4096]
    NH = len(BOUNDS) - 1

    with tile.TileContext(nc) as tc:
        with (
            tc.tile_pool(name="big", bufs=2 * NH) as big,
            tc.tile_pool(name="small", bufs=1) as small,
            tc.tile_pool(name="psum", bufs=2, space="PSUM") as psump,
        ):
            w1s = small.tile([128, O], F32)
            xs8 = small.tile([128, 2 * NH], F32)
            xs_all = small.tile([128, 2], F32)  # [p, T]: sum over all n
            rhs2 = small.tile([128, 4], F32)
            scp = small.tile([128, 8], F32)     # [o_p, m*4 + b]
            st0 = psump.tile([128, 4], F32)
            st1 = psump.tile([128, 4], F32)
            sts = [st0, st1]

            nc.vector.memset(rhs2[:, :], 0.0)

            # xk rows are flat (b*64 + c); partition-tile T covers b pair
            # (2T, 2T+1).  T-major emission: T=0's contraction overlaps
            # T=1's loads.  DMAs alternate across the two HWDGE rings
            # (SP / Activation); the weight load rides the SP ring after
            # T=0's first chunk so it's resident before the first matmul.
            ring = [nc.sync, nc.scalar, nc.gpsimd]
            for t in range(2):
                for h in range(NH):
                    lo, hi = BOUNDS[h], BOUNDS[h + 1]
                    xt = big.tile([128, hi - lo], F32)
                    ring[h % len(ring)].dma_start(
                        out=xt[:, :],
                        in_=xk[t * 128:(t + 1) * 128, lo:hi],
                    )
                    nc.vector.reduce_sum(
                        xs8[:, (t * NH + h):(t * NH + h) + 1], xt[:, :],
                        axis=mybir.AxisListType.X,
                    )
                if t == 0:
                    # Weights: w1tt[p, o] = W1[o, k*64 + p%64], replicated
                    # twice along partitions (each half serves one b of a
                    # pair of batches).
                    nc.sync.dma_start(out=w1s[:, :], in_=w1tt[:, :])

                nc.vector.reduce_sum(
                    xs_all[:, t:t + 1], xs8[:, NH * t:NH * (t + 1)],
                    axis=mybir.AxisListType.X,
                )
                # rhs2[:, 2t+j] = xs_all[:, t] masked to partition half j,
                # so the K=128 contraction only mixes rows of one b.
                nc.vector.tensor_copy(rhs2[0:64, 2 * t:2 * t + 1],
                                      xs_all[0:64, t:t + 1])
                nc.vector.tensor_copy(rhs2[64:128, 2 * t + 1:2 * t + 2],
                                      xs_all[64:128, t:t + 1])
                for m in range(2):
                    nc.tensor.matmul(
                        sts[m][:, 2 * t:2 * t + 2],
                        w1s[:, m * 128:(m + 1) * 128],
                        rhs2[:, 2 * t:2 * t + 2],
                        start=True, stop=True,
                    )

            for m in range(2):
                nc.vector.tensor_copy(scp[:, 4 * m:4 * m + 4], sts[m][:, :])
            nc.scalar.dma_start(out=out_ext[:, :], in_=scp[:, :])

    nc.compile()
    return nc


def _shard(x, W1):
    in_maps = []
    for k in range(N_CORES):
        xk = np.ascontiguousarray(
            x[:, k * CSH:(k + 1) * CSH, 0, :]
        ).reshape(ROWS, N)
        w1tt = np.ascontiguousarray(
            np.tile(W1[:, k * CSH:(k + 1) * CSH].T, (2, 1))
        )
        in_maps.append({"xk": xk, "w1tt": w1tt})
    return in_maps


def _assemble(spart_list):
    """Host gather: sum the per-core partial S, elu, broadcast along n."""
    ps = np.zeros((128, 8), dtype=np.float32)
    for sp in spart_list:
        ps += sp
    s_t = np.concatenate([ps[:, 0:4], ps[:, 4:8]], axis=0)  # [O, B]
    s = s_t.T  # [B, O]
    e = np.where(s > 0, s, np.expm1(np.minimum(s, 0))).astype(np.float32)
    full = np.broadcast_to(e[:, :, None, None], (B, O, 1, N))
    return np.ascontiguousarray(full, dtype=np.float32)


def kernel(x, W1, w2, bias_mat):
    x = np.ascontiguousarray(x, dtype=np.float32)
    W1 = np.ascontiguousarray(W1, dtype=np.float32)

    nc = _build()
    in_maps = _shard(x, W1)
    res = run_bass_kernel_spmd(nc, in_maps, core_ids=list(range(N_CORES)))
    return _assemble([res.results[k]["spart"] for k in range(N_CORES)])


if __name__ == "__main__":
    rng = np.random.default_rng(0)
    x = rng.standard_normal((B, C, 1, N), dtype=np.float32)
    W1 = (rng.standard_normal((O, C), dtype=np.float32) * 0.05)
    w2 = (rng.standard_normal((O,), dtype=np.float32) * 0.05)
    bias_mat = np.zeros((N, N), dtype=np.float32)
    out = kernel(x=x, W1=W1, w2=w2, bias_mat=bias_mat)
    print("out", out.shape, out.dtype, out[0, :4, 0, 0])


# revision 20
# speedup vs baseline: 1.0952x; 1.0952x over previous
"""Trainium2 Bass kernel for nn_Attn_head_40364102648200.

The reference computes a GAT-style attention head, but applies
softmax(..., axis=1) to a [B,1,N,N] tensor whose axis 1 has size 1 —
the softmax is over a singleton axis, so the attention coefficients are
identically 1.0 and the whole N x N logits/leaky-relu machinery is dead
code (for ANY input values).  The output reduces exactly to

    S[b,o]       = sum_c W1[o,c] * (sum_n x[b,c,0,n])
    out[b,o,0,n] = elu(S[b,o])            (broadcast along n)

The real work is streaming the 32 MB input x and reducing it over n
(4M adds), then a small channel contraction.  Strategy on 8
NeuronCores (channel-sharded SPMD, no cross-core collective):

  - core k reads x[:, k*64:(k+1)*64, 0, :]  (4 MB each, 1/8 of x),
    reduces over n on the Vector engine (input DMAs split across both
    HWDGE rings), and contracts its 64 channels with its W1 shard on
    the TensorEngine -> partial S_k [256, 4]
  - the host gather step sums the eight 4 KB partials (the cross-core
    reduce), applies elu to the 1024 S values, and broadcasts along n
    to materialize the full [4, 256, 1, 4096] output.

Keeping the 4 KB combine on the host instead of an on-device AllReduce
removes the all-core barrier; each core's NEFF execution is then
independent of the others' launch skew.
"""

import numpy as np

import concourse.bacc as bacc
import concourse.mybir as mybir
import concourse.tile as tile
from concourse.bass_utils import run_bass_kernel_spmd

F32 = mybir.dt.float32

N_CORES = 8
B, C, N, O = 4, 512, 4096, 256
CSH = C // N_CORES  # 64 channels per core
ROWS = B * CSH      # 256 flattened (b, c) rows per core


def _build():
    nc = bacc.Bacc(
        "TRN2",
        target_bir_lowering=False,
        debug=False,
        num_devices=N_CORES,
    )

    xk = nc.declare_dram_parameter("xk", [ROWS, N], F32, isOutput=False)
    w1tt = nc.declare_dram_parameter("w1tt", [128, O], F32, isOutput=False)
    # Partial S^T for this core's channel shard: [o_p, m*4 + b]
    out_ext = nc.declare_dram_parameter("spart", [128, 8], F32, isOutput=True)

    # Free-dim chunk boundaries per partition-tile: big chunks first, a
    # small final chunk so the last (serial) reduce is short.  Chunks
    # alternate across the two HWDGE rings (SP / Activation); gpsimd's
    # SWDGE path was measured slower (the engine blocks ~15us on the
    # transfer), so it is not used.
    BOUNDS = [0, 1792, 3584, 4096]
    NH = len(BOUNDS) - 1

    with tile.TileContext(nc) as tc:
        with (
            tc.tile_pool(name="big", bufs=2 * NH) as big,
            tc.tile_pool(name="small", bufs=1) as small,
            tc.tile_pool(name="psum", bufs=2, space="PSUM") as psump,
        ):
            w1s = small.tile([128, O], F32)
            xs8 = small.tile([128, 2 * NH], F32)
            xs_all = small.tile([128, 2], F32)  # [p, T]: sum over all n
            rhs2 = small.tile([128, 4], F32)
            scp = small.tile([128, 8], F32)     # [o_p, m*4 + b]
            st0 = psump.tile([128, 4], F32)
            st1 = psump.tile([128, 4], F32)
            sts = [st0, st1]

            nc.vector.memset(rhs2[:, :], 0.0)

            # xk rows are flat (b*64 + c); partition-tile T covers b pair
            # (2T, 2T+1).  T-major emission: T=0's contraction overlaps
            # T=1's loads.  DMAs alternate across the two HWDGE rings
            # (SP / Activation); the weight load rides the SP ring after
            # T=0's first chunk so it's resident before the first matmul.
            ring = [nc.sync, nc.scalar]
            for t in range(2):
                for h in range(NH):
                    lo, hi = BOUNDS[h], BOUNDS[h + 1]
                    xt = big.tile([128, hi - lo], F32)
                    ring[h % len(ring)].dma_start(
                        out=xt[:, :],
                        in_=xk[t * 128:(t + 1) * 128, lo:hi],
                    )
                    nc.vector.reduce_sum(
                        xs8[:, (t * NH + h):(t * NH + h) + 1], xt[:, :],
                        axis=mybir.AxisListType.X,
                    )
                if t == 0:
                    # Weights: w1tt[p, o] = W1[o, k*64 + p%64], replicated
                    # twice along partitions (each half serves one b of a
                    # pair of batches).
                    nc.sync.dma_start(out=w1s[:, :], in_=w1tt[:, :])

                nc.vector.reduce_sum(
                    xs_all[:, t:t + 1], xs8[:, NH * t:NH * (t + 1)],
                    axis=mybir.AxisListType.X,
                )
                # rhs2[:, 2t+j] = xs_all[:, t] masked to partition half j,
                # so the K=128 contraction only mixes rows of one b.
                nc.vector.tensor_copy(rhs2[0:64, 2 * t:2 * t + 1],
                                      xs_all[0:64, t:t + 1])
                nc.vector.tensor_copy(rhs2[64:128, 2 * t + 1:2 * t + 2],
                                      xs_all[64:128, t:t + 1])
                for m in range(2):
                    nc.tensor.matmul(
                        sts[m][:, 2 * t:2 * t + 2],
                        w1s[:, m * 128:(m + 1) * 128],
                        rhs2[:, 2 * t:2 * t + 2],
                        start=True, stop=True,
                    )

            for m in range(2):
                nc.vector.tensor_copy(scp[:, 4 * m:4 * m + 4], sts[m][:, :])
            nc.scalar.dma_start(out=out_ext[:, :], in_=scp[:, :])

    nc.compile()
    return nc


def _shard(x, W1):
    in_maps = []
    for k in range(N_CORES):
        xk = np.ascontiguousarray(
            x[:, k * CSH:(k + 1) * CSH, 0, :]
        ).reshape(ROWS, N)
        w1tt = np.ascontiguousarray(
            np.tile(W1[:, k * CSH:(k + 1) * CSH].T, (2, 1))
        )
        in_maps.append({"xk": xk, "w1tt": w1tt})
    return in_maps


def _assemble(spart_list):
    """Host gather: sum the per-core partial S, elu, broadcast along n."""
    ps = np.zeros((128, 8), dtype=np.float32)
    for sp in spart_list:
        ps += sp
    s_t = np.concatenate([ps[:, 0:4], ps[:, 4:8]], axis=0)  # [O, B]
    s = s_t.T  # [B, O]
    e = np.where(s > 0, s, np.expm1(np.minimum(s, 0))).astype(np.float32)
    full = np.broadcast_to(e[:, :, None, None], (B, O, 1, N))
    return np.ascontiguousarray(full, dtype=np.float32)


def kernel(x, W1, w2, bias_mat):
    x = np.ascontiguousarray(x, dtype=np.float32)
    W1 = np.ascontiguousarray(W1, dtype=np.float32)

    nc = _build()
    in_maps = _shard(x, W1)
    res = run_bass_kernel_spmd(nc, in_maps, core_ids=list(range(N_CORES)))
    return _assemble([res.results[k]["spart"] for k in range(N_CORES)])


if __name__ == "__main__":
    rng = np.random.default_rng(0)
    x = rng.standard_normal((B, C, 1, N), dtype=np.float32)
    W1 = (rng.standard_normal((O, C), dtype=np.float32) * 0.05)
    w2 = (rng.standard_normal((O,), dtype=np.float32) * 0.05)
    bias_mat = np.zeros((N, N), dtype=np.float32)
    out = kernel(x=x, W1=W1, w2=w2, bias_mat=bias_mat)
    print("out", out.shape, out.dtype, out[0, :4, 0, 0])


# revision 21
# speedup vs baseline: 1.1101x; 1.0136x over previous
"""Trainium2 Bass kernel for nn_Attn_head_40364102648200.

The reference computes a GAT-style attention head, but applies
softmax(..., axis=1) to a [B,1,N,N] tensor whose axis 1 has size 1 —
the softmax is over a singleton axis, so the attention coefficients are
identically 1.0 and the whole N x N logits/leaky-relu machinery is dead
code (for ANY input values).  The output reduces exactly to

    S[b,o]       = sum_c W1[o,c] * (sum_n x[b,c,0,n])
    out[b,o,0,n] = elu(S[b,o])            (broadcast along n)

The real work is streaming the 32 MB input x and reducing it over n
(4M adds), then a small channel contraction.  Strategy on 8
NeuronCores (channel-sharded SPMD, no cross-core collective):

  - core k reads x[:, k*64:(k+1)*64, 0, :]  (4 MB each, 1/8 of x),
    reduces over n on the Vector engine (input DMAs split across both
    HWDGE rings), and contracts its 64 channels with its W1 shard on
    the TensorEngine -> partial S_k [256, 4]
  - the host gather step sums the eight 4 KB partials (the cross-core
    reduce), applies elu to the 1024 S values, and broadcasts along n
    to materialize the full [4, 256, 1, 4096] output.

Keeping the 4 KB combine on the host instead of an on-device AllReduce
removes the all-core barrier; each core's NEFF execution is then
independent of the others' launch skew.
"""

import numpy as np

import concourse.bacc as bacc
import concourse.mybir as mybir
import concourse.tile as tile
from concourse.bass_utils import run_bass_kernel_spmd

F32 = mybir.dt.float32

N_CORES = 8
B, C, N, O = 4, 512, 4096, 256
CSH = C // N_CORES  # 64 channels per core
ROWS = B * CSH      # 256 flattened (b, c) rows per core


def _build():
    nc = bacc.Bacc(
        "TRN2",
        target_bir_lowering=False,
        debug=False,
        num_devices=N_CORES,
    )

    xk = nc.declare_dram_parameter("xk", [ROWS, N], F32, isOutput=False)
    w1tt = nc.declare_dram_parameter("w1tt", [128, O], F32, isOutput=False)
    # Partial S^T for this core's channel shard: [o_p, m*4 + b]
    out_ext = nc.declare_dram_parameter("spart", [128, 8], F32, isOutput=True)

    # Free-dim chunk boundaries per partition-tile: big chunks first, a
    # small final chunk so the last (serial) reduce is short.  Chunks
    # alternate across the two HWDGE rings (SP / Activation); gpsimd's
    # SWDGE path was measured slower (the engine blocks ~15us on the
    # transfer), so it is not used.
    BOUNDS = [0, 1792, 3584, 4096]
    NH = len(BOUNDS) - 1

    with tile.TileContext(nc) as tc:
        with (
            tc.tile_pool(name="big", bufs=2 * NH) as big,
            tc.tile_pool(name="small", bufs=1) as small,
            tc.tile_pool(name="psum", bufs=2, space="PSUM") as psump,
        ):
            w1s = small.tile([128, O], F32)
            xs8 = small.tile([128, 2 * NH], F32)
            xs_all = small.tile([128, 2], F32)  # [p, T]: sum over all n
            rhs2 = small.tile([128, 4], F32)
            scp = small.tile([128, 8], F32)     # [o_p, m*4 + b]
            st0 = psump.tile([128, 4], F32)
            st1 = psump.tile([128, 4], F32)
            sts = [st0, st1]

            nc.vector.memset(rhs2[:, :], 0.0)

            # xk rows are flat (b*64 + c); partition-tile T covers b pair
            # (2T, 2T+1).  T-major emission: T=0's contraction overlaps
            # T=1's loads.  DMAs alternate across the two HWDGE rings
            # (SP / Activation); the weight load rides the SP ring after
            # T=0's first chunk so it's resident before the first matmul.
            ring = [nc.sync, nc.scalar]
            for t in range(2):
                for h in range(NH):
                    lo, hi = BOUNDS[h], BOUNDS[h + 1]
                    xt = big.tile([128, hi - lo], F32)
                    ring[(t * NH + h) % len(ring)].dma_start(
                        out=xt[:, :],
                        in_=xk[t * 128:(t + 1) * 128, lo:hi],
                    )
                    nc.vector.reduce_sum(
                        xs8[:, (t * NH + h):(t * NH + h) + 1], xt[:, :],
                        axis=mybir.AxisListType.X,
                    )
                if t == 0:
                    # Weights: w1tt[p, o] = W1[o, k*64 + p%64], replicated
                    # twice along partitions (each half serves one b of a
                    # pair of batches).
                    nc.sync.dma_start(out=w1s[:, :], in_=w1tt[:, :])

                nc.vector.reduce_sum(
                    xs_all[:, t:t + 1], xs8[:, NH * t:NH * (t + 1)],
                    axis=mybir.AxisListType.X,
                )
                # rhs2[:, 2t+j] = xs_all[:, t] masked to partition half j,
                # so the K=128 contraction only mixes rows of one b.
                nc.vector.tensor_copy(rhs2[0:64, 2 * t:2 * t + 1],
                                      xs_all[0:64, t:t + 1])
                nc.vector.tensor_copy(rhs2[64:128, 2 * t + 1:2 * t + 2],
                                      xs_all[64:128, t:t + 1])
                for m in range(2):
                    nc.tensor.matmul(
                        sts[m][:, 2 * t:2 * t + 2],
                        w1s[:, m * 128:(m + 1) * 128],
                        rhs2[:, 2 * t:2 * t + 2],
                        start=True, stop=True,
                    )

            for m in range(2):
                nc.vector.tensor_copy(scp[:, 4 * m:4 * m + 4], sts[m][:, :])
            nc.scalar.dma_start(out=out_ext[:, :], in_=scp[:, :])

    nc.compile()
    return nc


def _shard(x, W1):
    in_maps = []
    for k in range(N_CORES):
        xk = np.ascontiguousarray(
            x[:, k * CSH:(k + 1) * CSH, 0, :]
        ).reshape(ROWS, N)
        w1tt = np.ascontiguousarray(
            np.tile(W1[:, k * CSH:(k + 1) * CSH].T, (2, 1))
        )
        in_maps.append({"xk": xk, "w1tt": w1tt})
    return in_maps


def _assemble(spart_list):
    """Host gather: sum the per-core partial S, elu, broadcast along n."""
    ps = np.zeros((128, 8), dtype=np.float32)
    for sp in spart_list:
        ps += sp
    s_t = np.concatenate([ps[:, 0:4], ps[:, 4:8]], axis=0)  # [O, B]
    s = s_t.T  # [B, O]
    e = np.where(s > 0, s, np.expm1(np.minimum(s, 0))).astype(np.float32)
    full = np.broadcast_to(e[:, :, None, None], (B, O, 1, N))
    return np.ascontiguousarray(full, dtype=np.float32)


def kernel(x, W1, w2, bias_mat):
    x = np.ascontiguousarray(x, dtype=np.float32)
    W1 = np.ascontiguousarray(W1, dtype=np.float32)

    nc = _build()
    in_maps = _shard(x, W1)
    res = run_bass_kernel_spmd(nc, in_maps, core_ids=list(range(N_CORES)))
    return _assemble([res.results[k]["spart"] for k in range(N_CORES)])


if __name__ == "__main__":
    rng = np.random.default_rng(0)
    x = rng.standard_normal((B, C, 1, N), dtype=np.float32)
    W1 = (rng.standard_normal((O, C), dtype=np.float32) * 0.05)
    w2 = (rng.standard_normal((O,), dtype=np.float32) * 0.05)
    bias_mat = np.zeros((N, N), dtype=np.float32)
    out = kernel(x=x, W1=W1, w2=w2, bias_mat=bias_mat)
    print("out", out.shape, out.dtype, out[0, :4, 0, 0])
